# revision 3
# baseline (speedup 1.0000x reference)
"""ColorRandomizer Trainium2 kernel: brightness/contrast/saturation/hue on 8 cores.

Data-parallel: 4 images per core. Per image (all big elementwise DVE ops use
tensor_scalar / scalar_tensor_tensor, which hit the DVE 4x perf mode at fp16):
  ph1  x1 = min(x*bf, 1)                     (Pool TS fp32->fp16; sums via DVE reduce)
  ph2  x2 = clamp01(cf*x1 + delta)           (DVE TS mult,add + TS max,min)
  ph3  x3 = clip01(sf*x2 + (1-sf)*gray(x2))  (DVE TS/STT chain, clip on Pool)
  ph4  HSV hue shift:
         maxc/minc/cr/masks/diffs/J via STT (4x),
         invc = exp(-ln(cr+eps)) on ACT,
         i6 = J*invc + 6hf,
         tent: B2 = ||i6+b_c| - 3| via ACT Abs with bias,
         out_c = min(max((B2-1)*cr, 0) + minc, maxc) via 3 STT ops,
       fp16->fp32 out conversion on ACT Copy, DMA per channel.
Flat 4-image software pipeline interleaves DVE/ACT/Pool streams.
Storage fp16, scalars/accums fp32.
"""
import sys

for _p in ("/opt/trn_rl_repo",):
    if _p not in sys.path:
        sys.path.append(_p)

import numpy as np
from concourse import bass, bacc, mybir, tile, bass_isa
from concourse.bass_utils import run_bass_kernel_spmd

F32 = mybir.dt.float32
F16 = mybir.dt.float16
I16 = mybir.dt.int16
OP = mybir.AluOpType
AF = mybir.ActivationFunctionType
AX = mybir.AxisListType

NIMG = 4          # images per core
H, W = 480, 640
NPIX = H * W      # 307200
F = NPIX // 128   # 2400 free elems per partition per plane
F3 = 3 * F
GRAY_W = (0.299, 0.587, 0.114)
EPS = 2e-5        # cr epsilon: keeps invc <= 5e4 (fits fp16)

TRACE = False     # test.py flips this for profiling
_CACHE = {}


def _build():
    nc = bacc.Bacc(None, target_bir_lowering=False)
    x_h = nc.declare_dram_parameter("x", [NIMG, 3, H, W], F32, isOutput=False)
    fac_h = nc.declare_dram_parameter("fac", [NIMG, 8], F32, isOutput=False)
    y_h = nc.declare_dram_parameter("y", [NIMG, 3, H, W], F32, isOutput=True)

    dma = nc.sync  # HWDGE

    # activation float biases must exist as const APs
    for v in (EPS, 0.0, -2.0, -4.0, -3.0):
        t = nc.alloc_sbuf_tensor(f"cst-{v}", [128, 1], F32)
        nc.gpsimd.memset(t.ap(), v)
        nc.const_aps.aps[(F32, v)] = t.ap()
    nc.all_engine_barrier()

    with tile.TileContext(nc) as tc:
        with tc.tile_pool(name="p", bufs=1) as pool:
            # broadcast per-image factors to all partitions once
            fac1 = pool.tile([1, NIMG * 8], F32)
            dma.dma_start(fac1[:], fac_h[:].flatten()[None, :])
            facb = pool.tile([128, NIMG * 8], F32)
            nc.gpsimd.partition_broadcast(facb[:], fac1[:], channels=128)

            def col(i, k):
                return facb[:, i * 8 + k : i * 8 + k + 1]

            def v3(ap):
                return ap.rearrange("p (c f) -> p c f", c=3)

            def v2(ap):
                return ap.rearrange("p (c f) -> p c f", c=2)

            def b3(ap_f):  # [128, F] -> [128, 3, F] broadcast
                return ap_f[:, None, :].broadcast_to([128, 3, F])

            def b2(ap_f):
                return ap_f[:, None, :].broadcast_to([128, 2, F])

            state = {}

            def emit_load(i):
                """DMA in + Pool brightness + DVE sums -> delta."""
                x1 = pool.tile([128, F3], F16, tag="x1", bufs=2)
                sums = pool.tile([128, 4], F32, tag="sums", bufs=2)
                for c in range(3):
                    xin = pool.tile([128, F], F32, tag="xin", bufs=2)
                    dma.dma_start(
                        xin[:],
                        x_h[i, c].flatten().rearrange("(p f) -> p f", p=128),
                    )
                    # brightness on Pool: x1 = min(bf*x, 1), fp32 -> fp16
                    nc.gpsimd.tensor_scalar(
                        x1[:, c * F : (c + 1) * F], xin[:],
                        col(i, 0), 1.0, OP.mult, OP.min,
                    )
                    # per-channel sums for the contrast mean (DVE free-dim reduce)
                    nc.vector.tensor_reduce(
                        sums[:, c : c + 1], x1[:, c * F : (c + 1) * F],
                        AX.X, OP.add,
                    )
                ws = pool.tile([128, 1], F32, tag="ws", bufs=2)
                nc.vector.tensor_scalar(ws[:], sums[:, 0:1], GRAY_W[0], None, OP.mult)
                nc.vector.scalar_tensor_tensor(ws[:], sums[:, 1:2], GRAY_W[1], ws[:], OP.mult, OP.add)
                nc.vector.scalar_tensor_tensor(ws[:], sums[:, 2:3], GRAY_W[2], ws[:], OP.mult, OP.add)
                ssum = pool.tile([128, 1], F32, tag="ssum", bufs=2)
                nc.gpsimd.partition_all_reduce(ssum[:], ws[:], 128, bass_isa.ReduceOp.add)
                delta = pool.tile([128, 1], F32, tag="delta", bufs=2)
                nc.vector.tensor_scalar(delta[:], ssum[:], col(i, 2), None, OP.mult)
                state[i] = [x1, delta]

            def emit_mainA(i):
                """contrast + saturation; x1 tile ends as x3 (clipped rgb)."""
                x1, delta = state[i]
                # x2 = clamp01(cf*x1 + delta), in-place
                nc.vector.tensor_scalar(x1[:], x1[:], col(i, 1), delta[:], OP.mult, OP.add)
                nc.vector.tensor_scalar(x1[:], x1[:], 0.0, 1.0, OP.max, OP.min)
                # gs = (1-sf) * gray(x2)
                g = pool.tile([128, F], F16, tag="g", bufs=2)
                nc.vector.tensor_scalar(g[:], x1[:, 0:F], col(i, 4), None, OP.mult)
                nc.vector.scalar_tensor_tensor(g[:], x1[:, F:2 * F], col(i, 5), g[:], OP.mult, OP.add)
                nc.vector.scalar_tensor_tensor(g[:], x1[:, 2 * F:3 * F], col(i, 6), g[:], OP.mult, OP.add)
                # x3u = sf*x2 + gs (in-place), then clip01 on Pool
                nc.vector.scalar_tensor_tensor(v3(x1[:]), v3(x1[:]), col(i, 3), b3(g[:]), OP.mult, OP.add)
                nc.gpsimd.tensor_scalar(x1[:], x1[:], 0.0, 1.0, OP.max, OP.min)

            def emit_headD(i):
                """maxc/minc/cr, masks, J (= cr * H6), invc on ACT."""
                x1, _ = state[i]
                r, gch, b = x1[:, 0:F], x1[:, F:2 * F], x1[:, 2 * F:3 * F]
                mmc = pool.tile([128, F3], F16, tag="mmc", bufs=2)
                MX, MN, CR = mmc[:, 0:F], mmc[:, F:2 * F], mmc[:, 2 * F:3 * F]
                nc.vector.scalar_tensor_tensor(MX, r, 1.0, gch, OP.mult, OP.max)
                nc.vector.scalar_tensor_tensor(MX, MX, 1.0, b, OP.mult, OP.max)
                nc.vector.scalar_tensor_tensor(MN, r, 1.0, gch, OP.mult, OP.min)
                nc.vector.scalar_tensor_tensor(MN, MN, 1.0, b, OP.mult, OP.min)
                nc.vector.scalar_tensor_tensor(CR, MN, -1.0, MX, OP.mult, OP.add)
                mk = pool.tile([128, 2 * F], I16, tag="mk", bufs=1)
                nc.vector.scalar_tensor_tensor(
                    v2(mk[:]), v2(x1[:, 0:2 * F]), 1.0, b2(MX), OP.mult, OP.is_ge,
                )
                dd = pool.tile([128, F3], F16, tag="dd", bufs=1)
                dgb, drg, dbr = dd[:, 0:F], dd[:, F:2 * F], dd[:, 2 * F:3 * F]
                nc.vector.scalar_tensor_tensor(dgb, b, -1.0, gch, OP.mult, OP.add)      # g-b
                nc.vector.scalar_tensor_tensor(drg, gch, -1.0, r, OP.mult, OP.add)      # r-g
                nc.vector.scalar_tensor_tensor(dbr, drg, -1.0, dgb, OP.mult, OP.subtract)  # b-r
                J = pool.tile([128, F], F16, tag="J", bufs=2)
                jg = pool.tile([128, F], F16, tag="jg", bufs=2)
                nc.vector.scalar_tensor_tensor(J[:], CR, 4.0, drg, OP.mult, OP.add)
                nc.vector.scalar_tensor_tensor(jg[:], CR, 2.0, dbr, OP.mult, OP.add)
                nc.vector.copy_predicated(J[:], mk[:, F:2 * F], jg[:])
                nc.vector.copy_predicated(J[:], mk[:, 0:F], dgb)
                # ACT: invc = exp(-ln(cr+eps)), fp16 (Ln/Exp adjacent: lc bufs=1)
                lc = pool.tile([128, F], F32, tag="lc", bufs=1)
                nc.scalar.activation(lc[:], CR, AF.Ln, bias=EPS)
                invc = pool.tile([128, F], F16, tag="invc", bufs=2)
                nc.scalar.activation(invc[:], lc[:], AF.Exp, scale=-1.0)
                state[i] = [x1, mmc, J, jg, invc]

            def emit_i6(i):
                x1, mmc, J, jg, invc = state[i]
                # i6 = J*invc + 6hf (reuse jg tile)
                i6 = jg
                nc.vector.scalar_tensor_tensor(i6[:], J[:], 1.0, invc[:], OP.mult, OP.mult)
                nc.vector.tensor_scalar(i6[:], i6[:], col(i, 7), None, OP.add)
                state[i] = [mmc, i6]

            def emit_tail(i):
                mmc, i6 = state[i]
                MX, MN, CR = mmc[:, 0:F], mmc[:, F:2 * F], mmc[:, 2 * F:3 * F]
                # tent on ACT: B1_c = |i6 + b_c| (b = 0,-2,-4), B2 = |B1 - 3|
                bt = pool.tile([128, F3], F16, tag="bt", bufs=2)
                for ci, bb in enumerate((0.0, -2.0, -4.0)):
                    nc.scalar.activation(bt[:, ci * F : (ci + 1) * F], i6[:], AF.Abs, bias=bb)
                nc.scalar.activation(bt[:], bt[:], AF.Abs, bias=-3.0)
                # out = min(max((B2-1)*cr, 0) + minc, maxc)  (DVE STT chain)
                nc.vector.scalar_tensor_tensor(v3(bt[:]), v3(bt[:]), -1.0, b3(CR), OP.add, OP.mult)
                nc.vector.scalar_tensor_tensor(v3(bt[:]), v3(bt[:]), 0.0, b3(MN), OP.max, OP.add)
                nc.vector.scalar_tensor_tensor(v3(bt[:]), v3(bt[:]), 1.0, b3(MX), OP.mult, OP.min)
                # fp16 -> fp32 on ACT, store per channel
                for c in range(3):
                    o32 = pool.tile([128, F], F32, tag="o32", bufs=3)
                    nc.scalar.activation(o32[:], bt[:, c * F : (c + 1) * F], AF.Copy)
                    dma.dma_start(
                        y_h[i, c].flatten().rearrange("(p f) -> p f", p=128),
                        o32[:],
                    )

            # flat software pipeline: pair-2 load/mainA fills pair-1 tail latency
            emit_load(0); emit_load(1)
            emit_mainA(0); emit_mainA(1)
            emit_headD(0); emit_headD(1)
            emit_i6(0); emit_i6(1)
            emit_load(2); emit_load(3)
            emit_tail(0); emit_tail(1)
            emit_mainA(2); emit_mainA(3)
            emit_headD(2); emit_headD(3)
            emit_i6(2); emit_i6(3)
            emit_tail(2); emit_tail(3)

    nc.finalize()
    return nc


def _get_nc():
    if "nc" not in _CACHE:
        _CACHE["nc"] = _build()
    return _CACHE["nc"]


def make_fac(bf, cf, sf, hf):
    return np.stack(
        [
            bf, cf, (1.0 - cf) / np.float32(NPIX), sf,
            GRAY_W[0] * (1.0 - sf), GRAY_W[1] * (1.0 - sf), GRAY_W[2] * (1.0 - sf),
            6.0 * hf,
        ],
        axis=1,
    ).astype(np.float32)


def kernel(x, brightness_f, contrast_f, saturation_f, hue_f, num_samples=1, **_):
    x = np.ascontiguousarray(np.asarray(x, dtype=np.float32))
    bf = np.asarray(brightness_f, np.float32)
    cf = np.asarray(contrast_f, np.float32)
    sf = np.asarray(saturation_f, np.float32)
    hf = np.asarray(hue_f, np.float32)
    fac = make_fac(bf, cf, sf, hf)

    nc = _get_nc()
    in_maps = [
        {"x": x[k * NIMG:(k + 1) * NIMG], "fac": fac[k * NIMG:(k + 1) * NIMG]}
        for k in range(8)
    ]
    res = run_bass_kernel_spmd(nc, in_maps, core_ids=list(range(8)), trace=TRACE)
    if TRACE:
        _CACHE["last"] = res
    out = np.concatenate([res.results[k]["y"] for k in range(8)], axis=0)
    return out.astype(np.float32)


# revision 4
# speedup vs baseline: 4.3599x; 4.3599x over previous
"""ColorRandomizer Trainium2 kernel: brightness/contrast/saturation/hue on 8 cores.

Data-parallel: 4 images per core. Per image:
  ph1  t = relu(1 - bf*x) on ACT (fp32->fp16, accum_out gives channel sums
       for the contrast mean for free); x1 = min(bf*x,1) = 1-t is never
       materialized -- ph2 consumes t directly.
  ph2  x2 = clamp01(-cf*t + (cf+delta))      (DVE TS 4x)
  ph3  x3 = clip01(sf*x2 + (1-sf)*gray(x2))  (DVE TS+TT)
  ph4  HSV hue shift:
         maxc/minc/cr/masks/diffs via TT (2x), J = cr*H6 via TS+TT+copy_pred,
         invc = exp(-ln(cr+eps)) on ACT,
         i6 = J*invc + 6hf,
         tent: B2_c = ||i6+b_c| - 3| via ACT Abs with bias,
         s = clamp01(B2-1) (TS), out = s*cr + minc (TT,TT; <= maxc by
         construction so no final min), fp16->fp32 out on ACT Copy.
Flat 4-image software pipeline interleaves DVE/ACT streams; gpsimd only does
tiny partition broadcast/all-reduce. Storage fp16, scalars/accums fp32.
"""
import sys

for _p in ("/opt/trn_rl_repo",):
    if _p not in sys.path:
        sys.path.append(_p)

import numpy as np
from concourse import bass, bacc, mybir, tile, bass_isa
from concourse.bass_utils import run_bass_kernel_spmd

F32 = mybir.dt.float32
F16 = mybir.dt.float16
I16 = mybir.dt.int16
OP = mybir.AluOpType
AF = mybir.ActivationFunctionType
AX = mybir.AxisListType

NIMG = 4          # images per core
H, W = 480, 640
NPIX = H * W      # 307200
F = NPIX // 128   # 2400 free elems per partition per plane
F3 = 3 * F
GRAY_W = (0.299, 0.587, 0.114)
EPS = 2e-5        # cr epsilon: keeps invc <= 5e4 (fits fp16)
NFAC = 10

TRACE = False     # test.py flips this for profiling
_CACHE = {}


def _build():
    nc = bacc.Bacc(None, target_bir_lowering=False)
    x_h = nc.declare_dram_parameter("x", [NIMG, 3, H, W], F32, isOutput=False)
    fac_h = nc.declare_dram_parameter("fac", [NIMG, NFAC], F32, isOutput=False)
    y_h = nc.declare_dram_parameter("y", [NIMG, 3, H, W], F32, isOutput=True)

    dma = nc.sync  # HWDGE

    # activation float biases must exist as const APs
    for v in (EPS, 0.0, -2.0, -4.0, -3.0, 1.0):
        t = nc.alloc_sbuf_tensor(f"cst-{v}", [128, 1], F32)
        nc.gpsimd.memset(t.ap(), v)
        nc.const_aps.aps[(F32, v)] = t.ap()
    nc.all_engine_barrier()

    with tile.TileContext(nc) as tc:
        with tc.tile_pool(name="p", bufs=1) as pool:
            # broadcast per-image factors to all partitions once
            fac1 = pool.tile([1, NIMG * NFAC], F32)
            dma.dma_start(fac1[:], fac_h[:].flatten()[None, :])
            facb = pool.tile([128, NIMG * NFAC], F32)
            nc.gpsimd.partition_broadcast(facb[:], fac1[:], channels=128)

            def col(i, k):
                return facb[:, i * NFAC + k : i * NFAC + k + 1]

            def v3(ap):
                return ap.rearrange("p (c f) -> p c f", c=3)

            def v2(ap):
                return ap.rearrange("p (c f) -> p c f", c=2)

            def b3(ap_f):  # [128, F] -> [128, 3, F] broadcast
                return ap_f[:, None, :].broadcast_to([128, 3, F])

            def b2(ap_f):
                return ap_f[:, None, :].broadcast_to([128, 2, F])

            state = {}

            def emit_load(i):
                """DMA in + ACT t = relu(1 - bf*x) with channel-sum accum -> delta."""
                x1 = pool.tile([128, F3], F16, tag="x1", bufs=2)
                sums = pool.tile([128, 4], F32, tag="sums", bufs=2)
                for c in range(3):
                    xin = pool.tile([128, F], F32, tag="xin", bufs=2)
                    dma.dma_start(
                        xin[:],
                        x_h[i, c].flatten().rearrange("(p f) -> p f", p=128),
                    )
                    # t_c = relu(1 - bf*x_c); sum(t_c) accumulates on the side
                    nc.scalar.activation(
                        x1[:, c * F : (c + 1) * F], xin[:],
                        AF.Relu, bias=1.0, scale=col(i, 0),
                        accum_out=sums[:, c : c + 1],
                    )
                # mean of gray(x1) = (NPIX - sum_c w_c*T_c)/NPIX  (sum w = 1)
                ws = pool.tile([128, 1], F32, tag="ws", bufs=2)
                nc.vector.tensor_scalar(ws[:], sums[:, 0:1], GRAY_W[0], None, OP.mult)
                nc.vector.scalar_tensor_tensor(ws[:], sums[:, 1:2], GRAY_W[1], ws[:], OP.mult, OP.add)
                nc.vector.scalar_tensor_tensor(ws[:], sums[:, 2:3], GRAY_W[2], ws[:], OP.mult, OP.add)
                ssum = pool.tile([128, 1], F32, tag="ssum", bufs=2)
                nc.gpsimd.partition_all_reduce(ssum[:], ws[:], 128, bass_isa.ReduceOp.add)
                # delta = (1-cf)*mean = (1-cf) - k2b*ssum;  bias2 = cf + delta
                bias2 = pool.tile([128, 1], F32, tag="bias2", bufs=2)
                nc.vector.tensor_scalar(bias2[:], ssum[:], col(i, 3), col(i, 9), OP.mult, OP.add)
                nc.vector.tensor_tensor(bias2[:], bias2[:], col(i, 2), OP.add)
                state[i] = [x1, bias2]

            def emit_mainA(i):
                """contrast + saturation; x1 tile: t -> x2 -> x3 (clipped rgb)."""
                x1, bias2 = state[i]
                # x2 = clamp01(-cf*t + (cf+delta)), in-place (TS 4x)
                nc.vector.tensor_scalar(x1[:], x1[:], col(i, 1), bias2[:], OP.mult, OP.add)
                nc.vector.tensor_scalar(x1[:], x1[:], 0.0, 1.0, OP.max, OP.min)
                # gs = (1-sf) * gray(x2): 3 TS + 2 TT
                g = pool.tile([128, F], F16, tag="g", bufs=1)
                gb = pool.tile([128, F], F16, tag="gb", bufs=1)
                nc.vector.tensor_scalar(g[:], x1[:, 0:F], col(i, 5), None, OP.mult)
                nc.vector.tensor_scalar(gb[:], x1[:, F:2 * F], col(i, 6), None, OP.mult)
                nc.vector.tensor_tensor(g[:], g[:], gb[:], OP.add)
                nc.vector.tensor_scalar(gb[:], x1[:, 2 * F:3 * F], col(i, 7), None, OP.mult)
                nc.vector.tensor_tensor(g[:], g[:], gb[:], OP.add)
                # x3 = clip01(sf*x2 + gs), in-place
                nc.vector.tensor_scalar(x1[:], x1[:], col(i, 4), None, OP.mult)
                nc.vector.tensor_tensor(v3(x1[:]), v3(x1[:]), b3(g[:]), OP.add)
                nc.vector.tensor_scalar(x1[:], x1[:], 0.0, 1.0, OP.max, OP.min)

            def emit_headD(i):
                """maxc/minc/cr, masks, J (= cr * H6), invc on ACT."""
                x1, _ = state[i]
                r, gch, b = x1[:, 0:F], x1[:, F:2 * F], x1[:, 2 * F:3 * F]
                mmc = pool.tile([128, F3], F16, tag="mmc", bufs=2)
                MX, MN, CR = mmc[:, 0:F], mmc[:, F:2 * F], mmc[:, 2 * F:3 * F]
                nc.vector.tensor_tensor(MX, r, gch, OP.max)
                nc.vector.tensor_tensor(MX, MX, b, OP.max)
                nc.vector.tensor_tensor(MN, r, gch, OP.min)
                nc.vector.tensor_tensor(MN, MN, b, OP.min)
                nc.vector.tensor_tensor(CR, MX, MN, OP.subtract)
                mk = pool.tile([128, 2 * F], I16, tag="mk", bufs=1)
                nc.vector.tensor_tensor(v2(mk[:]), v2(x1[:, 0:2 * F]), b2(MX), OP.is_ge)
                dd = pool.tile([128, F3], F16, tag="dd", bufs=1)
                dgb, drg, dbr = dd[:, 0:F], dd[:, F:2 * F], dd[:, 2 * F:3 * F]
                nc.vector.tensor_tensor(dgb, gch, b, OP.subtract)
                nc.vector.tensor_tensor(drg, r, gch, OP.subtract)
                nc.vector.tensor_tensor(dbr, b, r, OP.subtract)
                J = pool.tile([128, F], F16, tag="J", bufs=2)
                jg = pool.tile([128, F], F16, tag="jg", bufs=2)
                tmp = pool.tile([128, F], F16, tag="tmp", bufs=1)
                nc.vector.tensor_scalar(tmp[:], CR, 4.0, None, OP.mult)
                nc.vector.tensor_tensor(J[:], tmp[:], drg, OP.add)
                nc.vector.tensor_scalar(tmp[:], CR, 2.0, None, OP.mult)
                nc.vector.tensor_tensor(jg[:], tmp[:], dbr, OP.add)
                nc.vector.copy_predicated(J[:], mk[:, F:2 * F], jg[:])
                nc.vector.copy_predicated(J[:], mk[:, 0:F], dgb)
                # ACT: invc = exp(-ln(cr+eps)), fp16 (Ln/Exp adjacent: lc bufs=1)
                lc = pool.tile([128, F], F32, tag="lc", bufs=1)
                nc.scalar.activation(lc[:], CR, AF.Ln, bias=EPS)
                invc = pool.tile([128, F], F16, tag="invc", bufs=2)
                nc.scalar.activation(invc[:], lc[:], AF.Exp, scale=-1.0)
                state[i] = [x1, mmc, J, jg, invc]

            def emit_i6(i):
                x1, mmc, J, jg, invc = state[i]
                i6 = jg  # reuse
                nc.vector.tensor_tensor(i6[:], J[:], invc[:], OP.mult)
                nc.vector.tensor_scalar(i6[:], i6[:], col(i, 8), None, OP.add)
                state[i] = [mmc, i6]

            def emit_tail(i):
                mmc, i6 = state[i]
                MX, MN, CR = mmc[:, 0:F], mmc[:, F:2 * F], mmc[:, 2 * F:3 * F]
                # tent on ACT: B1_c = |i6 + b_c| (b = 0,-2,-4), B2 = |B1 - 3|
                bt = pool.tile([128, F3], F16, tag="bt", bufs=2)
                for ci, bb in enumerate((0.0, -2.0, -4.0)):
                    nc.scalar.activation(bt[:, ci * F : (ci + 1) * F], i6[:], AF.Abs, bias=bb)
                nc.scalar.activation(bt[:], bt[:], AF.Abs, bias=-3.0)
                # s = clamp01(B2 - 1); out = s*cr + minc (in [minc, maxc])
                nc.vector.tensor_scalar(bt[:], bt[:], -1.0, 0.0, OP.add, OP.max)
                nc.vector.tensor_scalar(bt[:], bt[:], 1.0, None, OP.min)
                nc.vector.tensor_tensor(v3(bt[:]), v3(bt[:]), b3(CR), OP.mult)
                nc.vector.tensor_tensor(v3(bt[:]), v3(bt[:]), b3(MN), OP.add)
                # fp16 -> fp32 on ACT, store per channel
                for c in range(3):
                    o32 = pool.tile([128, F], F32, tag="o32", bufs=2)
                    nc.scalar.activation(o32[:], bt[:, c * F : (c + 1) * F], AF.Copy)
                    dma.dma_start(
                        y_h[i, c].flatten().rearrange("(p f) -> p f", p=128),
                        o32[:],
                    )

            # flat software pipeline: pair-2 load/mainA fills pair-1 tail latency
            emit_load(0); emit_load(1)
            emit_mainA(0); emit_mainA(1)
            emit_headD(0); emit_headD(1)
            emit_i6(0); emit_i6(1)
            emit_load(2); emit_load(3)
            emit_tail(0); emit_tail(1)
            emit_mainA(2); emit_mainA(3)
            emit_headD(2); emit_headD(3)
            emit_i6(2); emit_i6(3)
            emit_tail(2); emit_tail(3)

    nc.finalize()
    return nc


def _get_nc():
    if "nc" not in _CACHE:
        _CACHE["nc"] = _build()
    return _CACHE["nc"]


def make_fac(bf, cf, sf, hf):
    one = np.float32(1.0)
    return np.stack(
        [
            -bf, -cf, cf, -(one - cf) / np.float32(NPIX), sf,
            GRAY_W[0] * (one - sf), GRAY_W[1] * (one - sf), GRAY_W[2] * (one - sf),
            6.0 * hf, (one - cf),
        ],
        axis=1,
    ).astype(np.float32)


def kernel(x, brightness_f, contrast_f, saturation_f, hue_f, num_samples=1, **_):
    x = np.ascontiguousarray(np.asarray(x, dtype=np.float32))
    bf = np.asarray(brightness_f, np.float32)
    cf = np.asarray(contrast_f, np.float32)
    sf = np.asarray(saturation_f, np.float32)
    hf = np.asarray(hue_f, np.float32)
    fac = make_fac(bf, cf, sf, hf)

    nc = _get_nc()
    in_maps = [
        {"x": x[k * NIMG:(k + 1) * NIMG], "fac": fac[k * NIMG:(k + 1) * NIMG]}
        for k in range(8)
    ]
    res = run_bass_kernel_spmd(nc, in_maps, core_ids=list(range(8)), trace=TRACE)
    if TRACE:
        _CACHE["last"] = res
    out = np.concatenate([res.results[k]["y"] for k in range(8)], axis=0)
    return out.astype(np.float32)


# revision 9
# speedup vs baseline: 4.5043x; 1.0331x over previous
"""ColorRandomizer Trainium2 kernel: brightness/contrast/saturation/hue on 8 cores.

Data-parallel: 4 images per core. Per image:
  ph1  t = relu(1 - bf*x) on ACT (fp32->fp16, accum_out gives channel sums
       for the contrast mean for free); x1 = min(bf*x,1) = 1-t is never
       materialized -- ph2 consumes t directly.
  ph2  x2 = clamp01(-cf*t + (cf+delta))      (DVE TS 4x)
  ph3  x3 = clip01(sf*x2 + (1-sf)*gray(x2))  (DVE TS+TT)
  ph4  HSV hue shift:
         maxc/minc/cr/masks/diffs via TT (2x), J = cr*H6 via TS+TT+copy_pred,
         invc = exp(-ln(cr+eps)) on ACT,
         i6 = J*invc + 6hf,
         tent: B2_c = ||i6+b_c| - 3| via ACT Abs with bias,
         s = clamp01(B2-1) (TS), out = s*cr + minc (TT,TT; <= maxc by
         construction so no final min), fp16->fp32 out on ACT Copy.
Flat 4-image software pipeline interleaves DVE/ACT streams; gpsimd only does
tiny partition broadcast/all-reduce. Storage fp16, scalars/accums fp32.
"""
import sys

for _p in ("/opt/trn_rl_repo",):
    if _p not in sys.path:
        sys.path.append(_p)

import numpy as np
from concourse import bass, bacc, mybir, tile, bass_isa
from concourse.bass_utils import run_bass_kernel_spmd

F32 = mybir.dt.float32
F16 = mybir.dt.float16
I16 = mybir.dt.int16
OP = mybir.AluOpType
AF = mybir.ActivationFunctionType
AX = mybir.AxisListType

NIMG = 4          # images per core
H, W = 480, 640
NPIX = H * W      # 307200
F = NPIX // 128   # 2400 free elems per partition per plane
F3 = 3 * F
GRAY_W = (0.299, 0.587, 0.114)
EPS = 2e-5        # cr epsilon: keeps invc <= 5e4 (fits fp16)
NFAC = 12

TRACE = False     # test.py flips this for profiling
_CACHE = {}


def _build():
    nc = bacc.Bacc(None, target_bir_lowering=False)
    x_h = nc.declare_dram_parameter("x", [NIMG, 3, H, W], F32, isOutput=False)
    fac_h = nc.declare_dram_parameter("fac", [NIMG, NFAC], F32, isOutput=False)
    y_h = nc.declare_dram_parameter("y", [NIMG, 3, H, W], F32, isOutput=True)

    dma = nc.sync  # HWDGE

    # activation float biases must exist as const APs
    for v in (EPS, 0.0, -2.0, -4.0, -3.0, 1.0):
        t = nc.alloc_sbuf_tensor(f"cst-{v}", [128, 1], F32)
        nc.gpsimd.memset(t.ap(), v)
        nc.const_aps.aps[(F32, v)] = t.ap()
    nc.all_engine_barrier()

    with tile.TileContext(nc) as tc:
        with tc.tile_pool(name="p", bufs=1) as pool:
            # broadcast per-image factors to all partitions once
            fac1 = pool.tile([1, NIMG * NFAC], F32)
            dma.dma_start(fac1[:], fac_h[:].flatten()[None, :])
            facb = pool.tile([128, NIMG * NFAC], F32)
            nc.gpsimd.partition_broadcast(facb[:], fac1[:], channels=128)

            def col(i, k):
                return facb[:, i * NFAC + k : i * NFAC + k + 1]

            def v3(ap):
                return ap.rearrange("p (c f) -> p c f", c=3)

            def v2(ap):
                return ap.rearrange("p (c f) -> p c f", c=2)

            def b3(ap_f):  # [128, F] -> [128, 3, F] broadcast
                return ap_f[:, None, :].broadcast_to([128, 3, F])

            def b2(ap_f):
                return ap_f[:, None, :].broadcast_to([128, 2, F])

            state = {}

            def emit_load(i):
                """DMA in + ACT t = relu(1 - bf*x) with channel-sum accum -> delta."""
                x1 = pool.tile([128, F3], F16, tag="x1", bufs=2)
                sums = pool.tile([128, 4], F32, tag="sums", bufs=2)
                for c in range(3):
                    xin = pool.tile([128, F], F32, tag="xin", bufs=2)
                    dma.dma_start(
                        xin[:],
                        x_h[i, c].flatten().rearrange("(p f) -> p f", p=128),
                    )
                    # t_c = relu(1 - bf*x_c); sum(t_c) accumulates on the side
                    nc.scalar.activation(
                        x1[:, c * F : (c + 1) * F], xin[:],
                        AF.Relu, bias=1.0, scale=col(i, 0),
                        accum_out=sums[:, c : c + 1],
                    )
                # mean of gray(x1) = (NPIX - sum_c w_c*T_c)/NPIX  (sum w = 1)
                ws = pool.tile([128, 1], F32, tag="ws", bufs=2)
                nc.vector.tensor_scalar(ws[:], sums[:, 0:1], GRAY_W[0], None, OP.mult)
                nc.vector.scalar_tensor_tensor(ws[:], sums[:, 1:2], GRAY_W[1], ws[:], OP.mult, OP.add)
                nc.vector.scalar_tensor_tensor(ws[:], sums[:, 2:3], GRAY_W[2], ws[:], OP.mult, OP.add)
                ssum = pool.tile([128, 1], F32, tag="ssum", bufs=2)
                nc.gpsimd.partition_all_reduce(ssum[:], ws[:], 128, bass_isa.ReduceOp.add)
                # delta = (1-cf)*mean = (1-cf) - k2b*ssum;  bias2 = cf + delta
                bias2 = pool.tile([128, 1], F32, tag="bias2", bufs=2)
                nc.vector.tensor_scalar(bias2[:], ssum[:], col(i, 3), col(i, 9), OP.mult, OP.add)
                nc.vector.tensor_tensor(bias2[:], bias2[:], col(i, 2), OP.add)
                state[i] = [x1, bias2]

            def emit_mainA(i):
                """contrast + saturation; x1 tile: t -> x2 -> x3 (clipped rgb)."""
                x1, bias2 = state[i]
                # x2 = clamp01(-cf*t + (cf+delta)), in-place (TS 4x)
                nc.vector.tensor_scalar(x1[:], x1[:], col(i, 1), bias2[:], OP.mult, OP.add)
                nc.vector.tensor_scalar(x1[:], x1[:], 0.0, 1.0, OP.max, OP.min)
                # gs = (1-sf) * gray(x2): 3 TS + 2 TT
                g = pool.tile([128, F], F16, tag="g", bufs=1)
                gb = pool.tile([128, F], F16, tag="gb", bufs=1)
                nc.vector.tensor_scalar(g[:], x1[:, 0:F], col(i, 5), None, OP.mult)
                nc.vector.tensor_scalar(gb[:], x1[:, F:2 * F], col(i, 6), None, OP.mult)
                nc.vector.tensor_tensor(g[:], g[:], gb[:], OP.add)
                nc.vector.tensor_scalar(gb[:], x1[:, 2 * F:3 * F], col(i, 7), None, OP.mult)
                nc.vector.tensor_tensor(g[:], g[:], gb[:], OP.add)
                # x3 = clip01(sf*x2 + gs), in-place
                nc.vector.tensor_scalar(x1[:], x1[:], col(i, 4), None, OP.mult)
                nc.vector.tensor_tensor(v3(x1[:]), v3(x1[:]), b3(g[:]), OP.add)
                nc.vector.tensor_scalar(x1[:], x1[:], 0.0, 1.0, OP.max, OP.min)

            def emit_headD(i):
                """maxc/minc/cr, masks, J (= cr * H6), invc on ACT."""
                x1, _ = state[i]
                r, gch, b = x1[:, 0:F], x1[:, F:2 * F], x1[:, 2 * F:3 * F]
                mmc = pool.tile([128, F3], F16, tag="mmc", bufs=2)
                MX, MN, CR = mmc[:, 0:F], mmc[:, F:2 * F], mmc[:, 2 * F:3 * F]
                nc.vector.tensor_tensor(MX, r, gch, OP.max)
                nc.vector.tensor_tensor(MX, MX, b, OP.max)
                nc.vector.tensor_tensor(MN, r, gch, OP.min)
                nc.vector.tensor_tensor(MN, MN, b, OP.min)
                nc.vector.tensor_tensor(CR, MX, MN, OP.subtract)
                mk = pool.tile([128, 2 * F], I16, tag="mk", bufs=1)
                nc.vector.tensor_tensor(v2(mk[:]), v2(x1[:, 0:2 * F]), b2(MX), OP.is_ge)
                dd = pool.tile([128, F3], F16, tag="dd", bufs=1)
                dgb, drg, dbr = dd[:, 0:F], dd[:, F:2 * F], dd[:, 2 * F:3 * F]
                nc.vector.tensor_tensor(dgb, gch, b, OP.subtract)
                nc.vector.tensor_tensor(drg, r, gch, OP.subtract)
                nc.vector.tensor_tensor(dbr, b, r, OP.subtract)
                J = pool.tile([128, F], F16, tag="J", bufs=2)
                jg = pool.tile([128, F], F16, tag="jg", bufs=2)
                tmp = pool.tile([128, F], F16, tag="tmp", bufs=1)
                nc.vector.tensor_scalar(tmp[:], CR, 4.0, None, OP.mult)
                nc.vector.tensor_tensor(J[:], tmp[:], drg, OP.add)
                nc.vector.tensor_scalar(tmp[:], CR, 2.0, None, OP.mult)
                nc.vector.tensor_tensor(jg[:], tmp[:], dbr, OP.add)
                nc.vector.copy_predicated(J[:], mk[:, F:2 * F], jg[:])
                nc.vector.copy_predicated(J[:], mk[:, 0:F], dgb)
                state[i] = [x1, mmc, J, jg]

            def emit_ln(i):
                """ACT Ln of cr (pair-batched with emit_exp to cut table loads)."""
                x1, mmc, J, jg = state[i]
                CR = mmc[:, 2 * F:3 * F]
                lc = pool.tile([128, F], F32, tag="lc", bufs=2)
                nc.scalar.activation(lc[:], CR, AF.Ln, bias=EPS)
                state[i] = [x1, mmc, J, jg, lc]

            def emit_exp(i):
                x1, mmc, J, jg, lc = state[i]
                invc = pool.tile([128, F], F16, tag="invc", bufs=2)
                nc.scalar.activation(invc[:], lc[:], AF.Exp, scale=-1.0)
                state[i] = [x1, mmc, J, jg, invc]

            def emit_i6(i):
                x1, mmc, J, jg, invc = state[i]
                i6 = jg  # reuse; 6hf rides the ACT B1 biases
                nc.vector.tensor_tensor(i6[:], J[:], invc[:], OP.mult)
                state[i] = [mmc, i6]

            def emit_tail(i):
                mmc, i6 = state[i]
                MX, MN, CR = mmc[:, 0:F], mmc[:, F:2 * F], mmc[:, 2 * F:3 * F]
                # tent on ACT: B1_c = |i6 + 6hf + b_c| (b = 0,-2,-4; biases host-side)
                bt = pool.tile([128, F3], F16, tag="bt", bufs=2)
                for ci, bcol in enumerate((8, 10, 11)):
                    nc.scalar.activation(bt[:, ci * F : (ci + 1) * F], i6[:], AF.Abs, bias=col(i, bcol))
                nc.scalar.activation(bt[:], bt[:], AF.Abs, bias=-3.0)
                # s = clamp01(B2 - 1); out = s*cr + minc (in [minc, maxc])
                nc.vector.tensor_scalar(bt[:], bt[:], -1.0, 0.0, OP.add, OP.max)
                nc.vector.tensor_scalar(bt[:], bt[:], 1.0, None, OP.min)
                nc.vector.tensor_tensor(v3(bt[:]), v3(bt[:]), b3(CR), OP.mult)
                nc.vector.tensor_tensor(v3(bt[:]), v3(bt[:]), b3(MN), OP.add)
                # fp16 -> fp32 on ACT, store per channel
                for c in range(3):
                    o32 = pool.tile([128, F], F32, tag="o32", bufs=2)
                    nc.scalar.activation(o32[:], bt[:, c * F : (c + 1) * F], AF.Copy)
                    dma.dma_start(
                        y_h[i, c].flatten().rearrange("(p f) -> p f", p=128),
                        o32[:],
                    )

            # flat software pipeline: pair-2 load/mainA fills pair-1 tail latency
            emit_load(0); emit_load(1)
            emit_mainA(0); emit_mainA(1)
            emit_headD(0); emit_headD(1)
            emit_ln(0); emit_ln(1); emit_exp(0); emit_exp(1)
            emit_i6(0); emit_i6(1)
            emit_load(2); emit_load(3)
            emit_mainA(2); emit_tail(0)
            emit_mainA(3); emit_tail(1)
            emit_headD(2); emit_headD(3)
            emit_ln(2); emit_ln(3); emit_exp(2); emit_exp(3)
            emit_i6(2); emit_i6(3)
            emit_tail(2); emit_tail(3)

    nc.finalize()
    return nc


def _get_nc():
    if "nc" not in _CACHE:
        _CACHE["nc"] = _build()
    return _CACHE["nc"]


def make_fac(bf, cf, sf, hf):
    one = np.float32(1.0)
    return np.stack(
        [
            -bf, -cf, cf, -(one - cf) / np.float32(NPIX), sf,
            GRAY_W[0] * (one - sf), GRAY_W[1] * (one - sf), GRAY_W[2] * (one - sf),
            6.0 * hf, (one - cf), 6.0 * hf - 2.0, 6.0 * hf - 4.0,
        ],
        axis=1,
    ).astype(np.float32)


def kernel(x, brightness_f, contrast_f, saturation_f, hue_f, num_samples=1, **_):
    x = np.ascontiguousarray(np.asarray(x, dtype=np.float32))
    bf = np.asarray(brightness_f, np.float32)
    cf = np.asarray(contrast_f, np.float32)
    sf = np.asarray(saturation_f, np.float32)
    hf = np.asarray(hue_f, np.float32)
    fac = make_fac(bf, cf, sf, hf)

    nc = _get_nc()
    in_maps = [
        {"x": x[k * NIMG:(k + 1) * NIMG], "fac": fac[k * NIMG:(k + 1) * NIMG]}
        for k in range(8)
    ]
    res = run_bass_kernel_spmd(nc, in_maps, core_ids=list(range(8)), trace=TRACE)
    if TRACE:
        _CACHE["last"] = res
    out = np.concatenate([res.results[k]["y"] for k in range(8)], axis=0)
    return out.astype(np.float32)
